# revision 2
# baseline (speedup 1.0000x reference)
"""Trainium2 Bass kernel for ContextQueryAttn (BiDAF-style trilinear attention).

Computes, per batch b (with context compacted to the NC2 unmasked rows):
    sim = sc[:,None] + sq[None,:] + (ctx*wm) @ query.T          (NC2, Lq)
    PT  = exp(sim^T + sq_bias[q])    [q, c]   (zero rows for masked q)
    Pc  = exp(cross)                 [c, q]   (e^{sc} folded into ctx rows)
    T   = Pc^T @ (e^{sc}[ctx|1])  -> col-softmax numerator + normalizer
    Tn  = T / colsum                 [q, d]
    A|B = PT^T @ [qe | Tn]           (NC2, 512)  one 512-col matmul per tile
    rs  = PT^T @ 1                   (NC2,)      row-softmax normalizer
    cm  = mq^T @ Tn                  (1, d)      sum of Tn over unmasked q
Host divides A|B by rs, reconstructs masked rows from cm / query-mean.

v5 engine budget (per core, 4 batches):
 - PE ~38.5us of column streaming (PT+Pc+T+AB+rs).
 - ACT: EXPs only (7 instrs/batch) + one small drain.
 - DVE: A|B drains (one [2,512]-wide copy per 2 ci) + Tn normalize.
 - DMA dispatch cost is ~4.7ns/descriptor on the issuing sequencer: spread
   across Sync (ctxT, ABo), Scalar (ctx2) and GpSimd/SWDGE (small tensors),
   with 2-batch grouped loads (one descriptor per partition line).
"""

import numpy as np
import ml_dtypes

import concourse.bass as bass
import concourse.tile as tile
from concourse import bacc, mybir
from concourse.bass_utils import run_bass_kernel_spmd

F32 = mybir.dt.float32
BF16 = mybir.dt.bfloat16
NPBF16 = ml_dtypes.bfloat16
EXP = mybir.ActivationFunctionType.Exp

B, LC, LQ, D = 32, 2048, 256, 256
NCORES = 8
BPC = B // NCORES          # batches per core
BG = BPC // 2              # 2-batch load groups per core
NKD = D // 128             # 2 contraction chunks over D
NQT = LQ // 128            # 2 query tiles
NEG = np.float32(-1e30)


def _build_kernel(tc, nc, ins, outs, NC2):
    import contextlib
    NT = NC2 // 128
    ctx = contextlib.ExitStack()

    sb = lambda name, bufs: ctx.enter_context(tc.tile_pool(name=name, bufs=bufs))
    psA = ctx.enter_context(tc.tile_pool(name="psA", bufs=3, space="PSUM"))
    psT = ctx.enter_context(tc.tile_pool(name="psT", bufs=2, space="PSUM"))

    p_ctxT = sb("pctxT", 2)
    p_ctx = sb("pctx", 2)
    p_qwm = sb("pqwm", 2)
    p_fv = sb("pfv", 2)
    p_mq = sb("pmq", 2)
    p_qeTn = sb("pqeTn", 2)
    p_pt = sb("ppt", 2)
    p_pc = sb("ppc", 2)
    p_cs = sb("pcs", 2)
    p_ast = sb("past", 2)
    p_rs = sb("prs", 2)
    p_cm = sb("pcm", 2)

    # PT column chunks (psum-tile sized) and Pc 4-ci groups
    pt_chunks = []
    off = 0
    while off < NC2:
        w = min(1024, NC2 - off)
        pt_chunks.append((off, w))
        off += w
    pc_groups = []
    i = 0
    while i < NT:
        pc_groups.append((i, min(4, NT - i)))
        i += 4
    ab_pairs = []
    i = 0
    while i < NT:
        ab_pairs.append((i, min(2, NT - i)))
        i += 2

    group_tiles = {}

    for b in range(BPC):
        g, j = divmod(b, 2)
        if j == 0:
            # ---- 2-batch grouped loads: one descriptor per partition ----
            ctxT_sb = p_ctxT.tile([128, 2, NKD, NC2], BF16, name="ctxT_sb")
            nc.sync.dma_start(out=ctxT_sb[:], in_=ins["ctxT2"][g])
            ctx_sb = p_ctx.tile([128, 2, NT, 258], BF16, name="ctx_sb")
            nc.scalar.dma_start(out=ctx_sb[:], in_=ins["ctx2"][g])
            qwm_sb = p_qwm.tile([128, 2, NKD, LQ], BF16, name="qwm_sb")
            nc.gpsimd.dma_start(out=qwm_sb[:], in_=ins["qwm2"][g])
            fv_sb = p_fv.tile([128, 2, NQT], F32, name="fv_sb")
            nc.gpsimd.dma_start(out=fv_sb[:], in_=ins["fvec"][g])
            mq_sb = p_mq.tile([128, 2, NQT], BF16, name="mq_sb")
            nc.gpsimd.dma_start(out=mq_sb[:], in_=ins["mq"][g])
            group_tiles = dict(ctxT=ctxT_sb, ctx=ctx_sb, qwm=qwm_sb,
                               fv=fv_sb, mq=mq_sb)
        ctxT_sb = group_tiles["ctxT"]
        ctx_sb = group_tiles["ctx"]
        qwm_sb = group_tiles["qwm"]
        fv_sb = group_tiles["fv"]
        mq_sb = group_tiles["mq"]

        # per-batch: [qe(256) | Tn(256) | ones(1)] rhs image for A|B + rowsum
        qeTn = p_qeTn.tile([128, NQT, 513], BF16, name="qeTn")
        nc.gpsimd.dma_start(out=qeTn[:], in_=ins["qe2"][b])

        # ---- phase 1: PT = exp(simT + sqb) [q, c'], Pc = exp(cross) [c', q],
        #      T accumulation braided behind the Pc EXPs ----
        PT_sb = p_pt.tile([128, NQT, NC2], BF16, name="PT_sb")
        Pc_sb = p_pc.tile([128, NT * LQ], BF16, name="Pc_sb")
        T_ps = [psT.tile([128, 512], F32, tag="psT", name=f"T_ps{qt}")
                for qt in range(NQT)]

        def emit_pt(qt, off, w):
            ps = psA.tile([128, 1024], F32, tag="psA", name="ps_pt")
            o2 = 0
            while o2 < w:
                cw = min(512, w - o2)
                for kd in range(NKD):
                    nc.tensor.matmul(
                        ps[:, o2:o2 + cw],
                        lhsT=qwm_sb[:, j, kd, bass.ts(qt, 128)],
                        rhs=ctxT_sb[:, j, kd, off + o2:off + o2 + cw],
                        start=(kd == 0), stop=(kd == NKD - 1))
                o2 += cw
            nc.scalar.activation(
                PT_sb[:, qt, off:off + w], ps[:, 0:w], EXP,
                bias=fv_sb[:, j, qt:qt + 1])

        def emit_pc(gi):
            ci0, w = pc_groups[gi]
            psc = psA.tile([128, 1024], F32, tag="psA", name="psc")
            for k in range(w):
                for kd in range(NKD):
                    nc.tensor.matmul(
                        psc[:, k * LQ:(k + 1) * LQ],
                        lhsT=ctxT_sb[:, j, kd, bass.ts(ci0 + k, 128)],
                        rhs=qwm_sb[:, j, kd, :],
                        start=(kd == 0), stop=(kd == NKD - 1))
            nc.scalar.activation(
                Pc_sb[:, ci0 * LQ:(ci0 + w) * LQ], psc[:, 0:w * LQ], EXP)

        def emit_t(ci):
            for qt in range(NQT):
                nc.tensor.matmul(
                    T_ps[qt][:, 0:258],
                    lhsT=Pc_sb[:, ci * LQ + qt * 128:ci * LQ + qt * 128 + 128],
                    rhs=ctx_sb[:, j, ci, :],
                    start=(ci == 0), stop=(ci == NT - 1))

        pt_units = [(qt, off, w) for qt in range(NQT)
                    for (off, w) in pt_chunks]
        npc = len(pc_groups)
        tq = []                 # T chains pending (their Pc EXP must be done)
        for i in range(max(len(pt_units), npc + 1)):
            if i < len(pt_units):
                emit_pt(*pt_units[i])
            if i < npc:
                emit_pc(i)
            if i >= 1 and i - 1 < npc:
                ci0, w = pc_groups[i - 1]
                tq.extend(range(ci0, ci0 + w))
            while len(tq) > 4:  # lag ~1 group behind the Pc EXPs
                emit_t(tq.pop(0))
        for ci in tq:
            emit_t(ci)

        # ---- T finalize: Tn = T * 1/colsum written into the qeTn image.
        #      masked-q rows are garbage; mq zeroes them out of cm, PT's
        #      zero rows annihilate them in B. ----
        csrec = p_cs.tile([128, NQT], F32, name="csrec")
        for qt in range(NQT):
            nc.vector.reciprocal(csrec[:, qt:qt + 1], T_ps[qt][:, 256:257])
            nc.vector.tensor_scalar_mul(
                qeTn[:, qt, 256:512], T_ps[qt][:, 0:256], csrec[:, qt:qt + 1])

        # ---- phase 2: A|B numerators (one 512-col matmul per tile),
        #      rowsum + Tn column-sum as cheap side matmuls ----
        ABst = p_ast.tile([128, NT * 512], BF16, name="ABst")
        rs_st = p_rs.tile([128, 16], F32, name="rs_st")
        cm_st = p_cm.tile([128, 256], F32, name="cm_st")
        rsm = psT.tile([128, 512], F32, tag="psT", name="rsm")

        for pi, (ci0, w) in enumerate(ab_pairs):
            pab = psA.tile([128, 1024], F32, tag="psA", name="pab")
            for k in range(w):
                ci = ci0 + k
                for qt in range(NQT):
                    nc.tensor.matmul(
                        pab[:, k * 512:k * 512 + 512],
                        lhsT=PT_sb[:, qt, bass.ts(ci, 128)],
                        rhs=qeTn[:, qt, 0:512],
                        start=(qt == 0), stop=(qt == NQT - 1))
                    nc.tensor.matmul(
                        rsm[:, ci:ci + 1],
                        lhsT=PT_sb[:, qt, bass.ts(ci, 128)],
                        rhs=qeTn[:, qt, 512:513],
                        start=(qt == 0), stop=(qt == NQT - 1))
            if w == 2:
                nc.vector.tensor_copy(
                    ABst[:, ci0 * 512:(ci0 + 2) * 512], pab[:, 0:1024])
            else:
                nc.scalar.copy(
                    ABst[:, ci0 * 512:(ci0 + 1) * 512], pab[:, 0:512])

        # cm = sum of Tn rows over unmasked q (masked-context B rows)
        for qt in range(NQT):
            nc.tensor.matmul(
                rsm[0:1, 256:512],
                lhsT=mq_sb[:, j, qt:qt + 1],
                rhs=qeTn[:, qt, 256:512],
                start=(qt == 0), stop=(qt == NQT - 1))

        nc.vector.tensor_copy(rs_st[:, 0:NT], rsm[:, 0:NT])
        nc.vector.tensor_copy(cm_st[0:1, :], rsm[0:1, 256:512])

        nc.sync.dma_start(out=outs["ABo"][b], in_=ABst[:])
        nc.gpsimd.dma_start(out=outs["rso"][b], in_=rs_st[:])
        nc.gpsimd.dma_start(out=outs["cmo"][b], in_=cm_st[0:1, :])

    ctx.close()


def build_program(NC2):
    NT = NC2 // 128
    nc = bacc.Bacc("TRN2", target_bir_lowering=False, debug=False,
                   num_devices=NCORES)
    ins = {
        "ctxT2": nc.dram_tensor("ctxT2", [BG, 128, 2, NKD, NC2], BF16,
                                kind="ExternalInput").ap(),
        "ctx2": nc.dram_tensor("ctx2", [BG, 128, 2, NT, 258], BF16,
                               kind="ExternalInput").ap(),
        "qwm2": nc.dram_tensor("qwm2", [BG, 128, 2, NKD, LQ], BF16,
                               kind="ExternalInput").ap(),
        "fvec": nc.dram_tensor("fvec", [BG, 128, 2, NQT], F32,
                               kind="ExternalInput").ap(),
        "mq": nc.dram_tensor("mq", [BG, 128, 2, NQT], BF16,
                             kind="ExternalInput").ap(),
        "qe2": nc.dram_tensor("qe2", [BPC, 128, NQT, 513], BF16,
                              kind="ExternalInput").ap(),
    }
    outs = {
        "ABo": nc.dram_tensor("ABo", [BPC, 128, NT * 512], BF16,
                              kind="ExternalOutput").ap(),
        "rso": nc.dram_tensor("rso", [BPC, 128, 16], F32,
                              kind="ExternalOutput").ap(),
        "cmo": nc.dram_tensor("cmo", [BPC, 1, 256], F32,
                              kind="ExternalOutput").ap(),
    }
    with tile.TileContext(nc) as tc:
        _build_kernel(tc, nc, ins, outs, NC2)
    nc.compile()
    return nc


def _aux(context_mask):
    """Per-batch unmasked-context indices and the padded compact size."""
    cm = np.asarray(context_mask).astype(bool)
    idx = [np.flatnonzero(~cm[b]) for b in range(cm.shape[0])]
    nmax = max((len(u) for u in idx), default=1)
    NC2 = max(256, ((int(nmax) + 127) // 128) * 128)
    return idx, NC2


def _img(a, p=128):
    """[N*p, X...] row-major -> SBUF image [p, N, X...] (row r = t*p + lane)."""
    n = a.shape[0] // p
    return np.ascontiguousarray(
        a.reshape((n, p) + a.shape[1:]).swapaxes(0, 1))


def host_prep(context, query, context_mask, query_mask, w0):
    """Host-side preprocessing: compact, shard, build device blobs."""
    f = np.float32
    context = np.asarray(context, dtype=f)
    query = np.asarray(query, dtype=f)
    w0 = np.asarray(w0, dtype=f)
    wc, wq, wm = w0[:D], w0[D:2 * D], w0[2 * D:]
    qmf = np.asarray(query_mask).astype(f)                  # (B, LQ)
    idx, NC2 = _aux(context_mask)
    NT = NC2 // 128

    sq = query @ wq                                         # (B, LQ)
    sq_bias = ((1.0 - qmf) * sq + qmf * NEG).astype(f)      # -1e30 on masked q
    qwmT = (query * wm).transpose(0, 2, 1)                  # (B, D, LQ) f32
    qe = np.zeros((B, LQ, 513), f)
    qe[:, :, 0:256] = query
    qe[:, :, 512] = 1.0

    in_maps = []
    for c in range(NCORES):
        m = {"ctxT2": np.zeros((BG, 128, 2, NKD, NC2), NPBF16),
             "ctx2": np.zeros((BG, 128, 2, NT, 258), NPBF16),
             "qwm2": np.empty((BG, 128, 2, NKD, LQ), NPBF16),
             "fvec": np.zeros((BG, 128, 2, NQT), f),
             "mq": np.zeros((BG, 128, 2, NQT), NPBF16),
             "qe2": np.empty((BPC, 128, NQT, 513), NPBF16)}
        for lb in range(BPC):
            b = c * BPC + lb
            g, jj = divmod(lb, 2)
            U = idx[b]
            n = len(U)
            cU = context[b][U]                              # (n, D)
            scU = cU @ wc                                   # (n,)
            ctxT_pad = np.zeros((D, NC2), f)
            ctxT_pad[:, :n] = cU.T
            m["ctxT2"][g, :, jj] = _img(ctxT_pad).astype(NPBF16)
            # ctx rows scaled by e^{sc[c]} (column-softmax weight); the
            # ones-col picks up the same factor => correct normalizer.
            ctx_pad = np.zeros((NC2, 258), f)
            ctx_pad[:n, :D] = cU
            ctx_pad[:n, D] = 1.0
            ctx_pad[:n] *= np.exp(scU, dtype=f)[:, None]
            m["ctx2"][g, :, jj] = _img(ctx_pad).astype(NPBF16)
            m["qwm2"][g, :, jj] = _img(qwmT[b]).astype(NPBF16)
            m["fvec"][g, :, jj] = sq_bias[b].reshape(NQT, 128).T
            m["mq"][g, :, jj] = (1.0 - qmf[b]).reshape(NQT, 128).T
            m["qe2"][lb] = _img(qe[b]).astype(NPBF16)
        in_maps.append(m)
    return in_maps


_cached_nc = {}


def get_program(NC2):
    if NC2 not in _cached_nc:
        _cached_nc[NC2] = build_program(NC2)
    return _cached_nc[NC2]


def run_on_hw(in_maps, **kwargs):
    NC2 = in_maps[0]["ctxT2"].shape[-1]
    nc = get_program(NC2)
    return run_bass_kernel_spmd(nc, in_maps, core_ids=list(range(NCORES)),
                                **kwargs)


def kernel(context, query, context_mask, query_mask, w0):
    f = np.float32
    context = np.asarray(context, dtype=f)
    query = np.asarray(query, dtype=f)
    w0 = np.asarray(w0, dtype=f)
    qmask = np.asarray(query_mask).astype(bool)
    idx, NC2 = _aux(context_mask)
    NT = NC2 // 128
    ctxmean = context.mean(1, dtype=np.float64).astype(f)   # (B, D)
    in_maps = host_prep(context, query, context_mask, query_mask, w0)
    res = run_on_hw(in_maps)

    A = np.empty((B, LC, D), f)
    Bm = np.empty((B, LC, D), f)
    cmask = np.asarray(context_mask).astype(bool)
    for c in range(NCORES):
        r = res.results[c]
        for lb in range(BPC):
            b = c * BPC + lb
            U = idx[b]
            n = len(U)
            ABr = r["ABo"][lb].astype(f).reshape(128, NT, 512).swapaxes(0, 1)
            ABr = ABr.reshape(NC2, 512)
            rs = r["rso"][lb][:, :NT].astype(f).T.reshape(NC2)
            cm = r["cmo"][lb][0].astype(f)                  # (256,)
            inv = 1.0 / rs[:n]
            A[b][U] = ABr[:n, 0:256] * inv[:, None]
            Bm[b][U] = ABr[:n, 256:512] * inv[:, None]
            nmq = float(qmask[b].sum())
            colmean = (cm + nmq * ctxmean[b]) / np.float32(LQ)
            mrow = cmask[b]
            A[b][mrow] = query[b].mean(0, dtype=np.float64).astype(f)
            Bm[b][mrow] = colmean
    return A, Bm
